# revision 73
# baseline (speedup 1.0000x reference)
"""Trainium2 Bass kernel for BaseModernHopfield energy + input-gradient.

Reference computation (full inputs x:[8,1024,512], W:[512,2048], beta:[1]):
    h = beta * (x @ W); e = relu(h)
    y = -0.5 * sum(e*e)                   # scalar
    g = grad_x(y) = -beta * e @ W.T       # [B, L, D]
    out = (y, g.sum(axis=0))              # (scalar, [L, D])

Key algebraic point: the required output is g.sum(0) and summation commutes
with the second matmul:  sum_b e_b @ W.T = (sum_b e_b) @ W.T.  So instead of
the hinted batch sharding (which would compute the full per-batch gradient and
psum it), we shard over the sequence dim L: core i owns rows l in
[128*i, 128*(i+1)) of ALL batches. Then

  phase 1: for its 128 rows, each core computes e_b = relu(beta * x_b @ W) for
           all 8 batches (same FLOPs as one full batch),
  local:   E = sum_b e_b  (pairwise add tree split across GpSimd + DVE;
           no cross-core communication at all),
  phase 2: g_rows = -beta * E @ W.T — ONE [128,2048]x[2048,512] matmul, 8x
           less PE work than the batch-parallel phase 2.

Per-batch e^2 row-sums (for the energy scalar) are taken before the E
reduction, fused as one DVE scalar_tensor_tensor per tile:
(h max 0) * relu(h) = relu(h)^2 with the row reduction via accum_out; a few
tiles go to ACT (Square + accum_out) to balance engine load. beta is folded
into the host-prepared inputs (beta*x and -beta*W.T), so no on-device scales
are needed. Host concatenates the 8 row slices of g and sums e^2 partials.

Device layout: each core receives xT3=x[:, rows_i, :].transpose(0,2,1)
([B,D,128]), W [D,H] and WT=W.T [H,D] (host transposes/casts), so no
on-device transposes are needed. Matmul operands are float32r (full fp32
storage; the PE runs 1 cycle/row for moving free dim >= 256 vs 4 cycles/row
for plain fp32; ~1.4e-4 result error) or bf16 via HOPFIELD_MM_DTYPE.

Throwaway matmuls on an early 1KB DMA warm the HAM-gated PE clock during the
input-DMA window; DMAs are ordered by first use so the PE starts early.

The walrus build in this container allows at most ONE semaphore wait per
instruction (and ZERO on self-loading f32r matmuls). Tile freely emits
multi-wait instructions, so `legalize_waits` post-processes the BIR: excess
waits are hoisted onto same-engine NoOp instructions inserted immediately
before the owner — identical semantics on in-order engine queues.
"""

import os
import sys

import numpy as np

for _p in ("/opt/trn_rl_repo",):
    if _p not in sys.path and os.path.isdir(_p):
        sys.path.insert(0, _p)

B, L, D, H = 8, 1024, 512, 2048
P = 128
NCORES = 8
LP = L // NCORES     # l-rows per core (128)
NFREE = 512          # matmul moving free dim (= one PSUM bank of fp32)
KD = D // P          # k-tiles contracting D (4)
KH = H // P          # k-tiles contracting H (16)
NQ = H // NFREE      # W quarter-tiles along H (4)
BH = B // 2          # batches per half-group (4); BH*LP = NFREE
NBH = 2              # batch half-groups

_cache = {}

# Filled by kernel() for the test harness: BassKernelResults of last run.
LAST_RESULTS = None


def _dep(a, b, sync, reason):
    from concourse.tile import add_dep_helper

    ra = a.ins if hasattr(a, "ins") else a
    rb = b.ins if hasattr(b, "ins") else b
    add_dep_helper(ra, rb, sync=sync, reason=reason)


def legalize_waits(nc):
    """Hoist semaphore waits so no instruction carries more than walrus's
    per-instruction wait budget (1; 0 for Matmult whose LW uop has no slot).

    A wait moved onto a NoOp scheduled immediately before the owner on the
    same engine queue is semantically identical: the queue is in-order, so
    the owner still cannot start before the wait is satisfied."""
    import concourse.mybir as mybir

    counter = [0]

    def budget(inst):
        return 0 if "Matmult" in type(inst).__name__ else 1

    for fn in nc.m.functions:
        for blk in fn.blocks:
            insts = blk.instructions
            out = []
            changed = False
            for inst in insts:
                si = inst.sync_info
                if si is not None:
                    waits = list(si.on_wait)
                    b = budget(inst)
                    if len(waits) > b:
                        keep = waits[len(waits) - b :] if b else []
                        hoist = waits[: len(waits) - b] if b else waits
                        for w in hoist:
                            counter[0] += 1
                            nop = mybir.InstNoOp(
                                name=f"waitnop_{counter[0]}", ins=[], outs=[]
                            )
                            nop.engine = inst.engine
                            nop.sync_info = mybir.SyncInfo(
                                on_wait=[w], on_update=[]
                            )
                            out.append(nop)
                        inst.sync_info = mybir.SyncInfo(
                            on_wait=keep, on_update=list(si.on_update)
                        )
                        changed = True
                out.append(inst)
            if changed:
                blk.instructions = out

    # audit
    for fn in nc.m.functions:
        for blk in fn.blocks:
            for inst in blk.instructions:
                si = inst.sync_info
                if si is None:
                    continue
                assert len(si.on_wait) <= budget(inst), (
                    f"{inst.name} {type(inst).__name__} still has "
                    f"{len(si.on_wait)} waits"
                )
    return counter[0]


def build_nc(beta: float, mm_dtype: str = "f32r", legalize: bool = True):
    import concourse.bass as bass
    import concourse.mybir as mybir
    import concourse.tile as tile

    f32 = mybir.dt.float32
    cdt = mybir.dt.float32r if mm_dtype == "f32r" else mybir.dt.bfloat16
    RELU = mybir.ActivationFunctionType.Relu
    COPY = mybir.ActivationFunctionType.Copy
    SQUARE = mybir.ActivationFunctionType.Square
    MULT = mybir.AluOpType.mult
    MAX = mybir.AluOpType.max
    ADD = mybir.AluOpType.add
    AX = mybir.AxisListType.X

    nc = bass.Bass()
    xT3 = nc.dram_tensor("xT3", [B, D, LP], cdt, kind="ExternalInput")
    W_ = nc.dram_tensor("W", [D, H], cdt, kind="ExternalInput")
    WT = nc.dram_tensor("WT", [H, D], cdt, kind="ExternalInput")
    g_ = nc.dram_tensor("g", [LP, D], f32, kind="ExternalOutput")
    e2 = nc.dram_tensor("e2", [P, 1], f32, kind="ExternalOutput")

    with tile.TileContext(nc) as tc:
        with (
            tc.tile_pool(name="big", bufs=1) as big,
            tc.tile_pool(name="psum", bufs=1, space="PSUM") as psp,
        ):
            W_sb = [
                [
                    big.tile([P, NFREE], cdt, tag=f"W{k}_{q}", name=f"W_sb{k}_{q}")
                    for q in range(NQ)
                ]
                for k in range(KD)
            ]
            WT_sb = [
                big.tile([P, D], cdt, tag=f"WT{k}", name=f"WT_sb{k}")
                for k in range(KH)
            ]
            # xT slice, [d partitions, batch, l]; split in batch-halves so the
            # first phase-1 groups depend on only half the xT bytes
            xT_sb = [
                [
                    big.tile([P, BH, LP], cdt, tag=f"xT{k}_{hb}",
                             name=f"xT_sb{k}_{hb}")
                    for hb in range(NBH)
                ]
                for k in range(KD)
            ]
            # per-batch relu activations for one h-tile: [h, b, l]
            e_all = [
                big.tile([P, B, LP], cdt, tag=f"eA{m}", name=f"e_all{m}")
                for m in range(KH)
            ]
            # batch-summed E^T tiles [h, l]
            ET = [
                big.tile([P, LP], cdt, tag=f"ET{m}", name=f"ET{m}")
                for m in range(KH)
            ]
            sq = big.tile([P, NFREE], f32, tag="sq")
            sqa = big.tile([P, NFREE], f32, tag="sqa")
            acc = big.tile([P, KH * NBH], f32, tag="acc")
            acc1 = big.tile([P, 1], f32, tag="acc1")
            g_sb = big.tile([P, D], f32, tag="g_sb")
            tmp4 = big.tile([P, BH, LP], cdt, tag="tmp4")
            tmp2 = big.tile([P, 2, LP], cdt, tag="tmp2")

            xT_r = xT3[:, :, :].rearrange("b (ko p) l -> ko p b l", p=P)
            W_r = W_[:, :].rearrange("(ko p) (q f) -> ko q p f", p=P, f=NFREE)
            WT_r = WT[:, :].rearrange("(ko p) d -> ko p d", p=P)

            warm_src = big.tile([P, 2], cdt, tag="warm_src")
            warm_f32 = big.tile([P, 2], f32, tag="warm_f32")
            nc.vector.memset(warm_f32[:, :], 1.0)
            nc.vector.tensor_copy(warm_src[:, :], warm_f32[:, :])

            # DMA order follows first-use order so the PE can start early:
            # phase-1 group (q=0,hb=0) needs xT[*][0] + W[*][q0]; then
            # xT[*][1]; then remaining W quarters; WT (phase 2) last.
            for ko in range(KD):
                nc.sync.dma_start(xT_sb[ko][0][:, :, :], xT_r[ko][:, 0:BH, :])
                nc.sync.dma_start(W_sb[ko][0][:, :], W_r[ko, 0])
            for ko in range(KD):
                nc.sync.dma_start(xT_sb[ko][1][:, :, :], xT_r[ko][:, BH:B, :])
            for q in range(1, NQ):
                for ko in range(KD):
                    nc.sync.dma_start(W_sb[ko][q][:, :], W_r[ko, q])
            for ko in range(KH):
                nc.sync.dma_start(WT_sb[ko][:, :], WT_r[ko])

            # ---- PE warm-up ----
            # The PE clock is HAM-gated to 1.2 GHz until ~3.4us of sustained
            # activity; run throwaway matmuls on an early tiny DMA so the PE
            # enters the input-DMA wait warm. Results are never read.
            warm_ps = psp.tile([P, NFREE], f32, tag="ps", name="warm_ps",
                               bufs=7)
            warm_last = None
            for wi in range(400):
                warm_last = nc.tensor.matmul(
                    warm_ps[0:2, 0:2],
                    lhsT=warm_src[:, 0:2],
                    rhs=warm_src[:, 0:2],
                    start=True,
                    stop=True,
                    skip_group_check=True,
                )

            # ---- Phase 1: e_all[m][:,b,:] = relu(beta * (W[:,m].T @ x_b.T)),
            # 4 batches per matmul group (free dim 4*128 = 512) ----
            p1_order = [
                (q * (KH // NQ) + mi, hb)
                for q in range(NQ)
                for hb in range(NBH)
                for mi in range(KH // NQ)
            ]
            p2_order = list(range(KH))
            # phase-2 accumulator: a dedicated PSUM bank that accumulates
            # E^T[k] @ WT[k] as soon as each E-tree completes (interleaved
            # with the phase-1 matmul stream)
            gp = psp.tile([P, NFREE], f32, tag="gp", name="ps2", bufs=1)
            def gp_mm(k, first, last):
                nc.tensor.matmul(
                    gp[:, :],
                    lhsT=ET[k][:, :],
                    rhs=WT_sb[k][:, :],
                    start=first,
                    stop=last,
                )

            for m, hb in p1_order:
                q, mi = divmod(m, KH // NQ)
                pt = psp.tile([P, NFREE], f32, tag="ps", name=f"ps1_{m}_{hb}",
                              bufs=7)
                for k in range(KD):
                    mm = nc.tensor.matmul(
                        pt[:, :],
                        lhsT=W_sb[k][q][:, mi * P : (mi + 1) * P],
                        rhs=xT_sb[k][hb][:, :, :],
                        start=(k == 0),
                        stop=(k == KD - 1),
                    )
                    if warm_last is not None:
                        _dep(mm, warm_last, False, "warmup before real MMs")
                        warm_last = None
                dst = e_all[m][:, hb * BH : (hb + 1) * BH, :]
                # beta is folded into the host-side inputs, so relu is a pure
                # max(h, 0)
                nc.scalar.activation(dst, pt[:, :], RELU)
                c = m * NBH + hb
                # The E-tree gates phase 2, so emit it right after the relu;
                # the e^2 squares are tail-path work and queue after it.
                if hb == NBH - 1:
                    # E^T[m] = sum_b e_b as a contiguous pairwise add tree:
                    # the big first add on the otherwise-idle GpSimd engine,
                    # the two small ones on the DVE (engine balance). The last
                    # trees gate phase 2, so run those fully on the DVE to
                    # skip the extra Pool pipeline hop.
                    eng1 = nc.vector if m >= KH - 3 else nc.gpsimd
                    with nc.allow_low_precision(reason="8-way batch sum"):
                        eng1.tensor_add(
                            tmp4[:, :, :],
                            e_all[m][:, 0:BH, :],
                            e_all[m][:, BH:B, :],
                        )
                        nc.vector.tensor_add(
                            tmp2[:, :, :], tmp4[:, 0:2, :], tmp4[:, 2:BH, :]
                        )
                        nc.vector.tensor_add(
                            ET[m][:, :], tmp2[:, 0, :], tmp2[:, 1, :]
                        )
                # per-batch e^2 row sums (e_all holds per-batch values, so
                # this may run any time after the relu):
                # (h max 0) * relu(h) == relu(h)^2 with the row reduction
                # fused via accum_out — one DVE op off the PSUM tile. A few
                # tiles go to ACT (Square + accum_out) for engine balance.
                if c % 8 == 3 or c >= 2 * KH - 6:
                    nc.scalar.activation(
                        sqa[:, :], dst, SQUARE, accum_out=acc[:, c : c + 1]
                    )
                else:
                    nc.vector.scalar_tensor_tensor(
                        out=sq[:, :],
                        in0=pt[:, :],
                        scalar=0.0,
                        in1=dst,
                        op0=MAX,
                        op1=MULT,
                        accum_out=acc[:, c : c + 1],
                    )

            # e^2 total only depends on phase 1 — finish it before phase 2 so
            # nothing but the final g store sits on the kernel tail.
            nc.vector.tensor_reduce(acc1[:, :], acc[:, :], axis=AX, op=ADD)
            nc.sync.dma_start(e2[:, :], acc1[:, :])

            for j, k in enumerate(p2_order):
                gp_mm(k, j == 0, j == KH - 1)

            # ---- Phase 2 epilogue ----
            # single copy + store: per-DMA semaphore propagation (~0.9us)
            # dominates, so one big store beats a pipelined split
            nc.scalar.copy(g_sb[:, :], gp[:, :])
            nc.sync.dma_start(g_[:, :], g_sb[:, :])

    if legalize:
        n_nops = legalize_waits(nc)
        if os.environ.get("HOPFIELD_DEBUG"):
            print(f"legalize_waits inserted {n_nops} nops")
    return nc


def _get_nc(beta: float):
    mm_dtype = os.environ.get("HOPFIELD_MM_DTYPE", "f32r")
    key = (float(beta), mm_dtype)
    if key not in _cache:
        _cache[key] = build_nc(float(beta), mm_dtype)
    return _cache[key]


def kernel(x: np.ndarray, W: np.ndarray, beta: np.ndarray):
    global LAST_RESULTS
    from concourse.bass_utils import run_bass_kernel_spmd

    mm_dtype = os.environ.get("HOPFIELD_MM_DTYPE", "f32r")
    x = np.asarray(x, dtype=np.float32)
    W = np.asarray(W, dtype=np.float32)
    beta_f = float(np.asarray(beta).reshape(-1)[0])

    nc = _get_nc(beta_f)

    if mm_dtype == "f32r":
        cast = lambda a: np.ascontiguousarray(a, dtype=np.float32)
    else:
        import ml_dtypes

        cast = lambda a: np.ascontiguousarray(a).astype(ml_dtypes.bfloat16)

    # beta is folded into the inputs: h' = (beta*x) @ W gives relu(beta*h),
    # and rhs = (-beta*W.T) gives g = -beta * E @ W.T directly.
    W_host = cast(W)
    WT_host = cast(-beta_f * W.T)
    in_maps = []
    for i in range(NCORES):
        xi = beta_f * x[:, i * LP : (i + 1) * LP, :].transpose(0, 2, 1)
        in_maps.append({"xT3": cast(xi), "W": W_host, "WT": WT_host})

    trace = bool(int(os.environ.get("HOPFIELD_TRACE", "0")))
    res = run_bass_kernel_spmd(
        nc,
        in_maps,
        core_ids=list(range(NCORES)),
        trace=trace,
    )
    LAST_RESULTS = res

    g = np.concatenate([r["g"] for r in res.results], axis=0)  # [L, D]
    e2_total = np.float64(0.0)
    for r in res.results:
        e2_total += np.float64(r["e2"].sum(dtype=np.float64))

    y = np.float32(-0.5 * e2_total)
    return (y, np.ascontiguousarray(g))
